# revision 4
# baseline (speedup 1.0000x reference)
"""Trainium2 Bass kernel for CRF mean log-likelihood (B=128, S=512, T=256).

Strategy: data-parallel over batch (16 sequences per core, 8 cores). The
forward-algorithm log-partition is computed in exponential space so the
per-step T x T logsumexp becomes a PE matmul:

    alpha_s = (E^T alpha_{s-1}) * exp(emit_s - delta)     E = exp(trans)

with a constant per-step shift delta ~= log(T) + 1/2 (keeps the state in a
narrow dynamic range; drift < +-6 in log space over a 255-step chain).

The chain is latency-bound (matmul -> DVE multiply -> matmul), so the
sequence is processed FROM BOTH ENDS simultaneously (meet in the middle):
  forward:  alpha_r = (E^T alpha_{r-1}) . ee_r          r = 1..255
  backward: u_s = (E u_{s+1}) . ee_s,                   s = 510..256
  Z        = (E^T alpha_255)^T u_256

Per-round structure (tuned from perfetto trace analysis):
  - ONE merged DVE tensor_tensor per chain-step ([128, 2, 16]) instead of
    two half ops: the Vector queue is strict-FIFO and its per-op issue
    interval (~100ns) was a main round-time component.
  - Program order per round = [fwd MMs][bwd TT][bwd MMs][fwd TT]: the two
    chains are phase-shifted half a round so no instruction reaches its
    FIFO head before its input semaphore is ready (no head-of-line
    blocking on either engine).
  - newt state tiles are explicit ping-pong pairs (no per-round tile-pool
    recycle semaphore ops on the Vector queue).
  - E / E^T are pre-exponentiated on host and shipped as bf16 (no device
    staging + activation at startup); emissions ship as bf16 (half DMA).
  - Startup DMAs are spread across engine queues (sync/scalar/vector/
    gpsimd) so their ~0.6us per-descriptor issue costs overlap.

The gold (numerator) score is O(B*S) gather work - computed on host.
"""
import numpy as np

B, S, T = 128, 512, 256
NCORES = 8
BPC = B // NCORES          # batch per core = 16
W = 128                    # steps per emissions chunk
DELTA = 6.045              # per-step log-space shift ~ log(256) + 0.5
KEEP_MM_WAITS = True       # skip bacc's move_matmul_waits_to_ldweights

_cache = {}


def build_nc(n_steps=S):
    import concourse.bass as bass
    import concourse.tile as tile
    from concourse import bacc, mybir
    from contextlib import ExitStack

    f32 = mybir.dt.float32
    bf16 = mybir.dt.bfloat16
    Exp = mybir.ActivationFunctionType.Exp

    assert n_steps >= 4
    R = (n_steps - 2) // 2           # rounds (fwd steps 1..R; bwd S-2..R+1)
    assert n_steps - 2 - R == R      # even split (S even)

    nc = bacc.Bacc()
    em = nc.declare_dram_parameter("em", [2, 128, n_steps, BPC], bf16,
                                   isOutput=False)
    ef = nc.declare_dram_parameter("ef", [2, 128, 2, 128], bf16,
                                   isOutput=False)
    eb = nc.declare_dram_parameter("eb", [2, 128, 2, 128], bf16,
                                   isOutput=False)
    stw = nc.declare_dram_parameter("stw", [2, 128, 1], f32, isOutput=False)
    enw = nc.declare_dram_parameter("enw", [2, 128, 1], f32, isOutput=False)
    out = nc.declare_dram_parameter("out", [1, BPC], f32, isOutput=True)

    with ExitStack() as ctx:
        tc = ctx.enter_context(tile.TileContext(nc))
        const = ctx.enter_context(tc.tile_pool(name="const", bufs=1))
        emf = ctx.enter_context(tc.tile_pool(name="emf", bufs=3))
        eef = ctx.enter_context(tc.tile_pool(name="eef", bufs=3))
        emb = ctx.enter_context(tc.tile_pool(name="emb", bufs=3))
        eeb = ctx.enter_context(tc.tile_pool(name="eeb", bufs=3))
        qpool = ctx.enter_context(tc.tile_pool(name="q", bufs=1, space="PSUM"))
        spool = ctx.enter_context(tc.tile_pool(name="s", bufs=1, space="PSUM"))

        # ---- one-time constants: DMAs spread across otherwise-idle queues
        # so their ~0.6us per-descriptor issue costs overlap at startup.
        Ef = []   # Ef[i][:, j, :] = lhsT block for fwd (E^T p)
        Eb = []   # Eb[i][:, j, :] = lhsT block for bwd (E u)
        for i in range(2):
            t = const.tile([128, 2, 128], bf16, tag=f"Ef{i}", name=f"Ef{i}")
            nc.sync.dma_start(out=t, in_=ef[i])
            Ef.append(t)
        for i in range(2):
            t = const.tile([128, 2, 128], bf16, tag=f"Eb{i}", name=f"Eb{i}")
            nc.scalar.dma_start(out=t, in_=eb[i])
            Eb.append(t)
        st_t = []
        ben = []
        for i in range(2):
            t = const.tile([128, 1], f32, tag=f"st{i}", name=f"st{i}")
            nc.sync.dma_start(out=t, in_=stw[i])
            st_t.append(t)
        for i in range(2):
            t = const.tile([128, 1], f32, tag=f"ben{i}", name=f"ben{i}")
            nc.sync.dma_start(out=t, in_=enw[i])  # host sent en - delta
            ben.append(t)
        dbias = const.tile([128, 1], f32, tag="dbias", name="dbias")
        nc.vector.memset(dbias, -DELTA)
        onesf = const.tile([128, 1], bf16, tag="onesf", name="onesf")
        nc.vector.memset(onesf, 1.0)

        # ping-pong state tiles (fixed allocations; no pool recycling)
        pf = [const.tile([128, 2, BPC], bf16, tag=f"pf{k}", name=f"pf{k}")
              for k in range(2)]
        pb = [const.tile([128, 2, BPC], bf16, tag=f"pb{k}", name=f"pb{k}")
              for k in range(2)]

        # ---- emissions chunk streaming (per direction) ----
        # Stream each chunk in 16-step pieces (DMA pair + exp ACT per
        # piece), ordered by consumption direction, so the first rounds'
        # ee slices are ready ~2us into the kernel. DMAs issue from the
        # (otherwise idle) GpSimd queue.
        def load_chunk(c, pool, eepool_, nm, descending=False,
                       first_only=False, tiles=None):
            s0, s1 = c * W, min(n_steps, (c + 1) * W)
            n = s1 - s0
            if tiles is None:
                t = pool.tile([128, 2, W, BPC], bf16, tag="emchunk",
                              name=f"em{nm}")
                te = eepool_.tile([128, 2, W, BPC], bf16, tag="eechunk",
                                  name=f"ee{nm}")
            else:
                t, te = tiles
            pieces = [(a, min(a + 16, n)) for a in range(0, n, 16)]
            if descending:
                pieces = pieces[::-1]
            if first_only:
                pieces = pieces[:1]
            elif tiles is not None:
                pieces = pieces[1:]
            for a, b in pieces:
                for i in range(2):
                    nc.gpsimd.dma_start(out=t[:, i, a:b, :],
                                        in_=em[i, :, s0 + a:s0 + b, :])
                nc.scalar.activation(te[:, :, a:b, :], t[:, :, a:b, :],
                                     Exp, bias=dbias)
            return t, te

        # ---- init: first pieces of the two boundary chunks + states ----
        cf = 0                       # forward chunk index
        cb = (n_steps - 1) // W      # backward chunk index
        tf = load_chunk(cf, emf, eef, "f0", first_only=True)
        same = (cb == cf)
        tb = tf if same else load_chunk(cb, emb, eeb, "b0", descending=True,
                                        first_only=True)
        em_f, ee_f = tf
        em_b, ee_b = tb

        for i in range(2):
            # alpha_0 = exp(st + em_0)   (no delta at step 0)
            nc.scalar.activation(pf[0][:, i, :], em_f[:, i, 0, :],
                                 Exp, bias=st_t[i])
            # u_{S-1} = exp(em_{S-1} + en - delta)
            nc.scalar.activation(pb[0][:, i, :],
                                 em_b[:, i, (n_steps - 1) % W, :],
                                 Exp, bias=ben[i])
        load_chunk(cf, emf, eef, "f0", tiles=tf)
        if not same:
            load_chunk(cb, emb, eeb, "b0", descending=True, tiles=tb)

        # PSUM accumulators (one tile per chain; both j-halves in one tile
        # so a single DVE op consumes them)
        qf = qpool.tile([128, 2, BPC], f32, tag="qf", name="qf")
        qb = qpool.tile([128, 2, BPC], f32, tag="qb", name="qb")

        def mm_group(q, E2, src):
            for j in range(2):
                for i in range(2):
                    nc.tensor.matmul(q[:, j, :], E2[i][:, j, :],
                                     src[:, i, :], start=(i == 0),
                                     stop=(i == 1))

        # bootstrap: q_b(0) = E u_{S-1}
        mm_group(qb, Eb, pb[0])

        # chunk bookkeeping: prefetch the next chunk half-way through the
        # current one (pools are triple-buffered), switch refs at bounds
        fwd_tiles = {cf: (em_f, ee_f)}
        bwd_tiles = {cb: (em_b, ee_b)}
        cf_hi, cb_lo = cf, cb
        for r in range(1, R + 1):
            sf = r                     # fwd step applied by this round's TT
            sb = n_steps - 1 - r       # bwd step applied by this round's TT
            ahead = min((sf + W // 2) // W, R // W)
            if ahead > cf_hi:
                cf_hi = ahead
                fwd_tiles[ahead] = load_chunk(ahead, emf, eef, f"f{ahead}")
            em_f, ee_f = fwd_tiles[sf // W]
            behind = max((sb - W // 2) // W, (R + 1) // W)
            if behind < cb_lo:
                cb_lo = behind
                bwd_tiles[behind] = load_chunk(behind, emb, eeb,
                                               f"b{behind}",
                                               descending=True)
            em_b, ee_b = bwd_tiles[sb // W]

            # (a) fwd MMs: q_f = E^T alpha_{r-1}
            mm_group(qf, Ef, pf[(r - 1) % 2])
            # (b) bwd TT: u_sb = q_b . ee_sb
            nc.vector.tensor_mul(pb[r % 2], qb, ee_b[:, :, sb % W, :])
            # (c) bwd MMs: q_b = E u_sb   (last round's is unused)
            if r < R:
                mm_group(qb, Eb, pb[r % 2])
            # (d) fwd TT: alpha_r = q_f . ee_r
            nc.vector.tensor_mul(pf[r % 2], qf, ee_f[:, :, sf % W, :])

        # ---- final: Z = (E^T alpha_R)^T u_{R+1} ----
        mm_group(qf, Ef, pf[R % 2])
        d = const.tile([128, 2, BPC], bf16, tag="d", name="d")
        nc.vector.tensor_mul(d, qf, pb[R % 2])
        fin = spool.tile([1, BPC], f32, tag="fin", name="fin")
        for i in range(2):
            nc.tensor.matmul(fin, onesf, d[:, i, :],
                             start=(i == 0), stop=(i == 1))
        res = const.tile([1, BPC], f32, tag="res", name="res")
        nc.vector.tensor_copy(res, fin)
        nc.sync.dma_start(out=out[0:1, :], in_=res)

    if KEEP_MM_WAITS:
        nc.move_matmul_waits_to_ldweights = lambda: None
    nc.compile()
    return nc


def _prep_inputs(emissions, transitions, start_transitions, end_transitions,
                 n_steps=S):
    """Host-side layout prep: per-core input maps."""
    import ml_dtypes
    bf16 = ml_dtypes.bfloat16
    emissions = np.asarray(emissions[:, :n_steps, :], dtype=np.float32)
    em_t = np.ascontiguousarray(
        emissions.transpose(2, 1, 0).astype(bf16)).reshape(
        2, 128, n_steps, B)  # [i, p, s, b]
    trm = np.asarray(transitions, np.float32)
    ef = np.ascontiguousarray(np.exp(trm).astype(bf16).reshape(2, 128, 2, 128))
    eb = np.ascontiguousarray(
        np.exp(trm.T).astype(bf16).reshape(2, 128, 2, 128))
    stw = np.ascontiguousarray(
        np.asarray(start_transitions, np.float32).reshape(2, 128, 1))
    enw = np.ascontiguousarray(
        (np.asarray(end_transitions, np.float32) - np.float32(DELTA))
        .reshape(2, 128, 1))
    in_maps = []
    for c in range(NCORES):
        in_maps.append({
            "em": np.ascontiguousarray(em_t[:, :, :, c * BPC:(c + 1) * BPC]),
            "ef": ef, "eb": eb, "stw": stw, "enw": enw,
        })
    return in_maps


def _gold_score_host(emissions, tags, mask, transitions, start_transitions,
                     end_transitions):
    emissions = np.asarray(emissions, np.float32)
    tags = np.asarray(tags, np.int64)
    m = np.asarray(mask, np.float32)
    emit = np.take_along_axis(emissions, tags[..., None], axis=2)[..., 0]
    trans = np.asarray(transitions, np.float32)[tags[:, :-1], tags[:, 1:]]
    score = (np.asarray(start_transitions, np.float32)[tags[:, 0]] + emit[:, 0]
             + ((emit[:, 1:] + trans) * m[:, 1:]).sum(axis=1))
    last_idx = np.asarray(mask, np.int64).sum(axis=1) - 1
    last_tags = np.take_along_axis(tags, last_idx[:, None], axis=1)[:, 0]
    return score + np.asarray(end_transitions, np.float32)[last_tags]


def _numpy_fallback(emissions, tags, mask, transitions, start_transitions,
                    end_transitions):
    """Reference-faithful numpy path (only used if mask is not all ones)."""
    em = np.asarray(emissions, np.float64)
    msk = np.asarray(mask, bool)
    trn = np.asarray(transitions, np.float64)
    alpha = np.asarray(start_transitions, np.float64)[None, :] + em[:, 0]
    for s in range(1, em.shape[1]):
        scores = alpha[:, :, None] + trn[None, :, :] + em[:, s][:, None, :]
        mx = scores.max(axis=1, keepdims=True)
        new = np.log(np.exp(scores - mx).sum(axis=1)) + mx[:, 0, :]
        alpha = np.where(msk[:, s][:, None], new, alpha)
    fin = alpha + np.asarray(end_transitions, np.float64)[None, :]
    mx = fin.max(axis=1, keepdims=True)
    logden = np.log(np.exp(fin - mx).sum(axis=1)) + mx[:, 0]
    gold = _gold_score_host(emissions, tags, mask, transitions,
                            start_transitions, end_transitions)
    return np.array(np.mean(gold - logden), dtype=np.float32)


def run_device(emissions, transitions, start_transitions, end_transitions,
               n_steps=S, trace=False, tmpdir=None):
    """Compile (cached) + run the Bass kernel; returns (logden[B], results)."""
    from concourse.bass_utils import run_bass_kernel_spmd
    key = n_steps
    if key not in _cache:
        _cache[key] = build_nc(n_steps)
    nc = _cache[key]
    in_maps = _prep_inputs(emissions, transitions, start_transitions,
                           end_transitions, n_steps)
    core_ids = list(range(NCORES))
    r = run_bass_kernel_spmd(nc, in_maps, core_ids, trace=trace, tmpdir=tmpdir)
    zprod = np.concatenate([np.asarray(r.results[c]["out"][0], np.float32)
                            for c in range(NCORES)])
    logden = np.log(zprod) + np.float32((n_steps - 1) * DELTA)
    return logden, r


def kernel(emissions, tags, mask, transitions, start_transitions,
           end_transitions):
    emissions = np.asarray(emissions)
    tags = np.asarray(tags)
    mask = np.asarray(mask)
    if not mask.all():
        return _numpy_fallback(emissions, tags, mask, transitions,
                               start_transitions, end_transitions)
    logden, _ = run_device(emissions, transitions, start_transitions,
                           end_transitions)
    gold = _gold_score_host(emissions, tags, mask, transitions,
                            start_transitions, end_transitions)
    return np.array(np.mean(gold - logden), dtype=np.float32)
